# revision 3
# baseline (speedup 1.0000x reference)
"""Causal self-attention on 8 trn2 NeuronCores.

Sharding (per the batch+head hint): core c handles batch b = c//2 (data
parallel) and head-group g = c%2 (8 of 16 heads; tensor-parallel slice of
w_qkv columns / w_out rows). Each core computes a full-batch-slice partial
of the output projection over its 512 head dims; the two partials per batch
are summed on gather (the "all-reduce after out_proj").

Kernel dataflow per core (S=2048 tokens, D=1024, 8 heads x 64):
  phase 1: x^T via PE transposes; qkvT = Wslice^T @ x^T (fp32r matmuls,
           512-token blocks so N=512 hides the fp32 LDWEIGHTS)
           -> qT, kT in bf16 [64h, S] layouts; v transposed back to natural
           [S, 64] bf16 with an appended ones column (denominator trick).
  phase 2: per head, exact-causal flash attention in transposed orientation,
           all bf16 on the PE: scoresT[k,q] = kT_chunk.T @ qT (scale folded
           into Wq on host), single lower-triangle additive mask on the
           diagonal chunks, exp with no max subtraction (scores are O(N(0,1)),
           fp32-safe), out_hT[dh,q] += [v|1].T @ probsT where the ones row
           accumulates the softmax denominator; normalize via reciprocal +
           gpsimd partition_broadcast.
  phase 3: partial out = oT.T @ Wout_slice (fp32r) per 128-token chunk.
"""
import numpy as np

B = 4
S = 2048
D = 1024
HG = 8           # heads per core
DH = 64
NCORES = 8
NB = S // 512    # 512-token blocks in phase 1
KC = D // 128    # contraction chunks over D
MC = (3 * 512) // 128  # qkv output chunks per core (12)

_CACHE = {}


def _build_nc():
    import concourse.bass as bass  # noqa
    import concourse.mybir as mybir
    import concourse.tile as tile
    from concourse import bacc
    from concourse.masks import make_identity

    F32 = mybir.dt.float32
    FR = mybir.dt.float32r
    BF = mybir.dt.bfloat16
    Exp = mybir.ActivationFunctionType.Exp

    nc = bacc.Bacc("TRN2", target_bir_lowering=False, debug=False,
                   enable_asserts=False, num_devices=NCORES)
    x_d = nc.dram_tensor("x", [S, D], F32, kind="ExternalInput")
    wqkv_d = nc.dram_tensor("wqkv", [D, 3 * 512], FR, kind="ExternalInput")
    wout_d = nc.dram_tensor("wout", [512, D], FR, kind="ExternalInput")
    masks_d = nc.dram_tensor("masks", [128, 512], F32, kind="ExternalInput")
    out_d = nc.dram_tensor("out", [S, D], F32, kind="ExternalOutput")

    with tile.TileContext(nc) as tc:
        with tc.tile_pool(name="persist", bufs=1) as persist:
            qT = persist.tile([128, 4 * S], BF)
            kT = persist.tile([128, 4 * S], BF)
            v1 = persist.tile([128, HG * 16 * 65], BF)
            ident = persist.tile([128, 128], F32)
            make_identity(nc, ident[:])
            ones128 = persist.tile([128, 128], F32)
            nc.gpsimd.memset(ones128[:], 1.0)
            tri0 = persist.tile([128, 512], F32)
            nc.sync.dma_start(tri0[:], masks_d.ap())
            # ones column of every [*, 65] v chunk
            nc.scalar.copy(
                v1[:].rearrange("p (c u) -> p c u", u=65)[:, :, 64:65],
                ones128[:].rearrange("p (c u) -> p c u", u=1),
            )

            # ---------------- phase 1: qkv projection ----------------
            with tc.tile_pool(name="wqkv", bufs=1) as wq_pool, \
                 tc.tile_pool(name="xin", bufs=3) as xin_pool, \
                 tc.tile_pool(name="xT", bufs=2) as xT_pool, \
                 tc.tile_pool(name="vst", bufs=2) as vst_pool, \
                 tc.tile_pool(name="ps_tp", bufs=2, space="PSUM") as ps_tp, \
                 tc.tile_pool(name="ps_mm", bufs=3, space="PSUM") as ps_mm, \
                 tc.tile_pool(name="ps_vt", bufs=2, space="PSUM") as ps_vt:
                w_sb = wq_pool.tile([128, KC * 1536], FR)
                nc.sync.dma_start(
                    w_sb[:].rearrange("p (k n) -> p k n", k=KC),
                    wqkv_d.ap().rearrange("(k p) n -> p k n", p=128),
                )
                for tb in range(NB):
                    xT_sb = xT_pool.tile([128, KC * 512], FR, tag="xT")
                    for t in range(4):
                        x_sb = xin_pool.tile([128, D], F32, tag="x")
                        nc.sync.dma_start(
                            x_sb[:], x_d[(tb * 4 + t) * 128:(tb * 4 + t + 1) * 128, :])
                        for ki in range(KC):
                            tp = ps_tp.tile([128, 128], F32, tag="tp")
                            nc.tensor.transpose(
                                tp[:], x_sb[:, ki * 128:(ki + 1) * 128], ident[:])
                            nc.scalar.copy(
                                xT_sb[:, ki * 512 + t * 128: ki * 512 + t * 128 + 128],
                                tp[:])
                    for m in range(MC):
                        acc = ps_mm.tile([128, 512], F32, tag="acc")
                        for ki in range(KC):
                            nc.tensor.matmul(
                                acc[:],
                                w_sb[:, ki * 1536 + m * 128: ki * 1536 + (m + 1) * 128],
                                xT_sb[:, ki * 512:(ki + 1) * 512],
                                start=(ki == 0), stop=(ki == KC - 1))
                        if m < 4:
                            nc.scalar.copy(
                                qT[:, m * S + tb * 512: m * S + tb * 512 + 512], acc[:])
                        elif m < 8:
                            nc.scalar.copy(
                                kT[:, (m - 4) * S + tb * 512: (m - 4) * S + tb * 512 + 512],
                                acc[:])
                        else:
                            vst = vst_pool.tile([128, 512], F32, tag="vst")
                            nc.vector.tensor_copy(vst[:], acc[:])
                            for t in range(4):
                                pvt = ps_vt.tile([128, 128], F32, tag="pvt")
                                nc.tensor.transpose(
                                    pvt[:], vst[:, t * 128:(t + 1) * 128], ident[:])
                                sck = tb * 4 + t
                                h0 = 2 * (m - 8)
                                nc.vector.tensor_copy(
                                    v1[:, (h0 * 16 + sck) * 65:(h0 * 16 + sck) * 65 + 64],
                                    pvt[:, 0:64])
                                nc.vector.tensor_copy(
                                    v1[:, ((h0 + 1) * 16 + sck) * 65:((h0 + 1) * 16 + sck) * 65 + 64],
                                    pvt[:, 64:128])

            # ---------------- phases 2+3 tensors ----------------
            with tc.tile_pool(name="attnbig", bufs=1) as attnbig:
              oT = attnbig.tile([128, 4 * S], FR)
              # ---------------- phase 2: causal attention ----------------
              with tc.tile_pool(name="probs", bufs=10) as pr_pool, \
                   tc.tile_pool(name="recip", bufs=2) as rc_pool, \
                   tc.tile_pool(name="rbc", bufs=2) as rb_pool, \
                   tc.tile_pool(name="ps_sc", bufs=3, space="PSUM") as ps_sc, \
                   tc.tile_pool(name="ps_out", bufs=1, space="PSUM") as ps_out:
                  for h in range(HG):
                      r = h // 2
                      po = 64 * (h % 2)
                      out_tiles = [
                          ps_out.tile([65, 512], F32, tag=f"oj{j}", name=f"outps_h{h}_j{j}")
                          for j in range(4)
                      ]
                      probs_tiles = {}

                      def emit_scores(s):
                          for j in range(s // 4, 4):
                              lo = max(128 * s - 512 * j, 0)
                              n = 512 - lo
                              scp = ps_sc.tile([128, 512], F32, tag="sc",
                                               name=f"sc_h{h}_s{s}_j{j}")
                              nc.tensor.matmul(
                                  scp[:, :n],
                                  kT[po:po + 64, r * S + s * 128: r * S + s * 128 + 128],
                                  qT[po:po + 64, r * S + 512 * j + lo: r * S + 512 * (j + 1)],
                                  start=True, stop=True)
                              if 128 * s >= 512 * j:
                                  nc.vector.tensor_add(
                                      scp[:, :n], scp[:, :n], tri0[:, :n])
                              pr = pr_pool.tile([128, 512], BF, tag="probs",
                                                name=f"pr_h{h}_s{s}_j{j}")
                              nc.scalar.activation(pr[:, :n], scp[:, :n], Exp)
                              probs_tiles[(s, j)] = pr

                      def emit_out(s):
                          for j in range(s // 4, 4):
                              lo = max(128 * s - 512 * j, 0)
                              n = 512 - lo
                              pr = probs_tiles.pop((s, j))
                              nc.tensor.matmul(
                                  out_tiles[j][:, lo:512],
                                  v1[:, (h * 16 + s) * 65:(h * 16 + s) * 65 + 65],
                                  pr[:, :n],
                                  start=(s == 0), stop=(s == 4 * j + 3))

                      emit_scores(0)
                      for s in range(1, 16):
                          emit_scores(s)
                          emit_out(s - 1)
                      emit_out(15)

                      for j in range(4):
                          rc = rc_pool.tile([1, 512], F32, tag="rc")
                          nc.vector.reciprocal(rc[:], out_tiles[j][64:65, :])
                          rb = rb_pool.tile([64, 512], F32, tag="rb")
                          nc.gpsimd.partition_broadcast(rb[:], rc[:])
                          nc.vector.tensor_mul(
                              oT[po:po + 64, r * S + 512 * j: r * S + 512 * j + 512],
                              out_tiles[j][0:64, :], rb[:])

              # ---------------- phase 3: output projection ----------------
              with tc.tile_pool(name="wout", bufs=1) as wo_pool, \
                   tc.tile_pool(name="ostage", bufs=4) as ost_pool, \
                   tc.tile_pool(name="ps_o", bufs=4, space="PSUM") as ps_o:
                  wout_sb = wo_pool.tile([128, 4 * D], FR)
                  nc.sync.dma_start(
                      wout_sb[:].rearrange("p (k n) -> p k n", k=4),
                      wout_d.ap().rearrange("(k p) n -> p k n", p=128),
                  )
                  for m in range(S // 128):
                      for half in range(2):
                          pso = ps_o.tile([128, 512], F32, tag="pso")
                          for k in range(4):
                              nc.tensor.matmul(
                                  pso[:],
                                  oT[:, k * S + m * 128: k * S + m * 128 + 128],
                                  wout_sb[:, k * D + half * 512: k * D + half * 512 + 512],
                                  start=(k == 0), stop=(k == 3))
                          ost = ost_pool.tile([128, 512], F32, tag="ost")
                          nc.vector.tensor_copy(ost[:], pso[:])
                          nc.sync.dma_start(
                              out_d[m * 128:(m + 1) * 128, half * 512:(half + 1) * 512],
                              ost[:])
    nc.compile()
    return nc


def _make_masks():
    # tri0[p, c] = 0 if c >= p else -1e10 (lower-triangle additive mask for
    # diagonal chunks whose moving window starts at the chunk diagonal)
    p = np.arange(128)[:, None]
    c = np.arange(512)[None, :]
    return np.where(c >= p, 0.0, -1e10).astype(np.float32)


def _make_in_maps(x, w_qkv, w_out):
    masks = _make_masks()
    scale = np.float32(DH ** -0.5)
    in_maps = []
    for c in range(NCORES):
        g = c % 2
        wq = w_qkv[:, g * 512:(g + 1) * 512] * scale
        wk = w_qkv[:, D + g * 512: D + (g + 1) * 512]
        wv = w_qkv[:, 2 * D + g * 512: 2 * D + (g + 1) * 512]
        in_maps.append({
            "x": np.ascontiguousarray(x[c // 2]),
            "wqkv": np.ascontiguousarray(np.concatenate([wq, wk, wv], axis=1)),
            "wout": np.ascontiguousarray(w_out[g * 512:(g + 1) * 512, :]),
            "masks": masks,
        })
    return in_maps


def kernel(x, w_qkv, w_out):
    from concourse.bass_utils import run_bass_kernel_spmd

    x = np.asarray(x, dtype=np.float32)
    w_qkv = np.asarray(w_qkv, dtype=np.float32)
    w_out = np.asarray(w_out, dtype=np.float32)
    assert x.shape == (B, S, D) and w_qkv.shape == (D, 3 * D) and w_out.shape == (D, D)

    if "nc" not in _CACHE:
        _CACHE["nc"] = _build_nc()
    nc = _CACHE["nc"]

    in_maps = _make_in_maps(x, w_qkv, w_out)
    res = run_bass_kernel_spmd(nc, in_maps, core_ids=list(range(NCORES)),
                               trace=False)
    out = np.empty((B, S, D), dtype=np.float32)
    for b in range(B):
        out[b] = res.results[2 * b]["out"] + res.results[2 * b + 1]["out"]
    return out


# revision 5
# speedup vs baseline: 1.0702x; 1.0702x over previous
"""Causal self-attention on 8 trn2 NeuronCores.

Sharding (per the batch+head hint): core c handles batch b = c//2 (data
parallel) and head-group g = c%2 (8 of 16 heads; tensor-parallel slice of
w_qkv columns / w_out rows). Each core computes a full-batch-slice partial
of the output projection over its 512 head dims; the two partials per batch
are summed on gather (the "all-reduce after out_proj").

Kernel dataflow per core (S=2048 tokens, D=1024, 8 heads x 64):
  phase 1: x^T via PE transposes; qkvT = Wslice^T @ x^T (fp32r matmuls,
           512-token blocks so N=512 hides the fp32 LDWEIGHTS)
           -> qT, kT in bf16 [64h, S] layouts; v transposed back to natural
           [S, 64] bf16 with an appended ones column (denominator trick).
  phase 2: per head, exact-causal flash attention in transposed orientation,
           all bf16 on the PE: scoresT[k,q] = kT_chunk.T @ qT (scale folded
           into Wq on host), single lower-triangle additive mask on the
           diagonal chunks, exp with no max subtraction (scores are O(N(0,1)),
           fp32-safe), out_hT[dh,q] += [v|1].T @ probsT where the ones row
           accumulates the softmax denominator; normalize via reciprocal +
           gpsimd partition_broadcast.
  phase 3: partial out = oT.T @ Wout_slice (fp32r) per 128-token chunk.
"""
import numpy as np

B = 4
S = 2048
D = 1024
HG = 8           # heads per core
DH = 64
NCORES = 8
NB = S // 512    # 512-token blocks in phase 1
KC = D // 128    # contraction chunks over D
MC = (3 * 512) // 128  # qkv output chunks per core (12)

_CACHE = {}


def _build_nc():
    import concourse.bass as bass  # noqa
    import concourse.mybir as mybir
    import concourse.tile as tile
    from concourse import bacc
    from concourse.masks import make_identity

    F32 = mybir.dt.float32
    FR = mybir.dt.float32r
    BF = mybir.dt.bfloat16
    Exp = mybir.ActivationFunctionType.Exp

    nc = bacc.Bacc("TRN2", target_bir_lowering=False, debug=False,
                   enable_asserts=False, num_devices=NCORES)
    x_d = nc.dram_tensor("x", [S, D], F32, kind="ExternalInput")
    wqkv_d = nc.dram_tensor("wqkv", [D, 3 * 512], FR, kind="ExternalInput")
    wout_d = nc.dram_tensor("wout", [512, D], FR, kind="ExternalInput")
    masks_d = nc.dram_tensor("masks", [128, 512], F32, kind="ExternalInput")
    out_d = nc.dram_tensor("out", [S, D], F32, kind="ExternalOutput")

    with tile.TileContext(nc) as tc:
        with tc.tile_pool(name="persist", bufs=1) as persist:
            qT = persist.tile([128, 4 * S], BF)
            kT = persist.tile([128, 4 * S], BF)
            v1 = persist.tile([128, HG * 16 * 65], BF)
            ident = persist.tile([128, 128], F32)
            make_identity(nc, ident[:])
            ones128 = persist.tile([128, 128], F32)
            nc.gpsimd.memset(ones128[:], 1.0)
            tri0 = persist.tile([128, 512], F32)
            nc.sync.dma_start(tri0[:], masks_d.ap())
            # ones column of every [*, 65] v chunk
            nc.scalar.copy(
                v1[:].rearrange("p (c u) -> p c u", u=65)[:, :, 64:65],
                ones128[:].rearrange("p (c u) -> p c u", u=1),
            )

            # ---------------- phase 1: qkv projection ----------------
            with tc.tile_pool(name="wqkv", bufs=1) as wq_pool, \
                 tc.tile_pool(name="xin", bufs=2) as xin_pool, \
                 tc.tile_pool(name="xT", bufs=2) as xT_pool, \
                 tc.tile_pool(name="vst", bufs=2) as vst_pool, \
                 tc.tile_pool(name="ps_tp", bufs=2, space="PSUM") as ps_tp, \
                 tc.tile_pool(name="ps_mm", bufs=3, space="PSUM") as ps_mm, \
                 tc.tile_pool(name="ps_vt", bufs=2, space="PSUM") as ps_vt:
                w_sb = wq_pool.tile([128, KC * 1536], FR)
                nc.sync.dma_start(
                    w_sb[:].rearrange("p (k n) -> p k n", k=KC),
                    wqkv_d.ap().rearrange("(k p) n -> p k n", p=128),
                )
                for tb in range(NB):
                    xT_sb = xT_pool.tile([128, KC * 512], FR, tag="xT")
                    x_tiles = []
                    for t in range(4):
                        x_sb = xin_pool.tile([128, D], F32, tag=f"x{t}")
                        nc.sync.dma_start(
                            x_sb[:], x_d[(tb * 4 + t) * 128:(tb * 4 + t + 1) * 128, :])
                        x_tiles.append(x_sb)
                    for ki in range(KC):
                        tp = ps_tp.tile([128, 512], F32, tag="tp")
                        for t in range(4):
                            nc.tensor.transpose(
                                tp[:, t * 128:(t + 1) * 128],
                                x_tiles[t][:, ki * 128:(ki + 1) * 128], ident[:])
                        nc.vector.tensor_copy(
                            xT_sb[:, ki * 512:(ki + 1) * 512], tp[:])
                    for m in range(MC):
                        acc = ps_mm.tile([128, 512], F32, tag="acc")
                        for ki in range(KC):
                            nc.tensor.matmul(
                                acc[:],
                                w_sb[:, ki * 1536 + m * 128: ki * 1536 + (m + 1) * 128],
                                xT_sb[:, ki * 512:(ki + 1) * 512],
                                start=(ki == 0), stop=(ki == KC - 1))
                        if m < 4:
                            nc.scalar.copy(
                                qT[:, m * S + tb * 512: m * S + tb * 512 + 512], acc[:])
                        elif m < 8:
                            nc.scalar.copy(
                                kT[:, (m - 4) * S + tb * 512: (m - 4) * S + tb * 512 + 512],
                                acc[:])
                        else:
                            vst = vst_pool.tile([128, 512], F32, tag="vst")
                            nc.vector.tensor_copy(vst[:], acc[:])
                            for t in range(4):
                                pvt = ps_vt.tile([128, 128], F32, tag="pvt")
                                nc.tensor.transpose(
                                    pvt[:], vst[:, t * 128:(t + 1) * 128], ident[:])
                                sck = tb * 4 + t
                                h0 = 2 * (m - 8)
                                nc.vector.tensor_copy(
                                    v1[:, (h0 * 16 + sck) * 65:(h0 * 16 + sck) * 65 + 64],
                                    pvt[:, 0:64])
                                nc.vector.tensor_copy(
                                    v1[:, ((h0 + 1) * 16 + sck) * 65:((h0 + 1) * 16 + sck) * 65 + 64],
                                    pvt[:, 64:128])

            # ---------------- phases 2+3 tensors ----------------
            with tc.tile_pool(name="attnbig", bufs=1) as attnbig:
              oT = attnbig.tile([128, 4 * S], FR)
              # ---------------- phase 2: causal attention ----------------
              with tc.tile_pool(name="probs", bufs=10) as pr_pool, \
                   tc.tile_pool(name="recip", bufs=2) as rc_pool, \
                   tc.tile_pool(name="rbc", bufs=2) as rb_pool, \
                   tc.tile_pool(name="ps_sc", bufs=3, space="PSUM") as ps_sc, \
                   tc.tile_pool(name="ps_out", bufs=1, space="PSUM") as ps_out:
                  for h in range(HG):
                      r = h // 2
                      po = 64 * (h % 2)
                      out_tiles = [
                          ps_out.tile([65, 512], F32, tag=f"oj{j}", name=f"outps_h{h}_j{j}")
                          for j in range(4)
                      ]
                      probs_tiles = {}

                      def emit_scores(s):
                          for j in range(s // 4, 4):
                              lo = max(128 * s - 512 * j, 0)
                              n = 512 - lo
                              scp = ps_sc.tile([128, 512], F32, tag="sc",
                                               name=f"sc_h{h}_s{s}_j{j}")
                              nc.tensor.matmul(
                                  scp[:, :n],
                                  kT[po:po + 64, r * S + s * 128: r * S + s * 128 + 128],
                                  qT[po:po + 64, r * S + 512 * j + lo: r * S + 512 * (j + 1)],
                                  start=True, stop=True)
                              if 128 * s >= 512 * j:
                                  nc.vector.tensor_add(
                                      scp[:, :128], scp[:, :128], tri0[:, :128])
                              pr = pr_pool.tile([128, 512], BF, tag="probs",
                                                name=f"pr_h{h}_s{s}_j{j}")
                              nc.scalar.activation(pr[:, :n], scp[:, :n], Exp)
                              probs_tiles[(s, j)] = pr

                      def normalize(j):
                          rc = rc_pool.tile([1, 512], F32, tag="rc")
                          nc.vector.reciprocal(rc[:], out_tiles[j][64:65, :])
                          rb = rb_pool.tile([64, 512], F32, tag="rb")
                          nc.gpsimd.partition_broadcast(rb[:], rc[:])
                          nc.vector.tensor_mul(
                              oT[po:po + 64, r * S + 512 * j: r * S + 512 * j + 512],
                              out_tiles[j][0:64, :], rb[:])

                      def emit_out(s):
                          for j in range(s // 4, 4):
                              lo = max(128 * s - 512 * j, 0)
                              n = 512 - lo
                              pr = probs_tiles.pop((s, j))
                              nc.tensor.matmul(
                                  out_tiles[j][:, lo:512],
                                  v1[:, (h * 16 + s) * 65:(h * 16 + s) * 65 + 65],
                                  pr[:, :n],
                                  start=(s == 0), stop=(s == 4 * j + 3))
                              if s == 4 * j + 3:
                                  normalize(j)

                      emit_scores(0)
                      for s in range(1, 16):
                          emit_scores(s)
                          emit_out(s - 1)
                      emit_out(15)

              # ---------------- phase 3: output projection ----------------
              with tc.tile_pool(name="wout", bufs=1) as wo_pool, \
                   tc.tile_pool(name="ostage", bufs=4) as ost_pool, \
                   tc.tile_pool(name="ps_o", bufs=4, space="PSUM") as ps_o:
                  wout_sb = wo_pool.tile([128, 4 * D], FR)
                  nc.sync.dma_start(
                      wout_sb[:].rearrange("p (k n) -> p k n", k=4),
                      wout_d.ap().rearrange("(k p) n -> p k n", p=128),
                  )
                  for m in range(S // 128):
                      for half in range(2):
                          pso = ps_o.tile([128, 512], F32, tag="pso")
                          for k in range(4):
                              nc.tensor.matmul(
                                  pso[:],
                                  oT[:, k * S + m * 128: k * S + m * 128 + 128],
                                  wout_sb[:, k * D + half * 512: k * D + half * 512 + 512],
                                  start=(k == 0), stop=(k == 3))
                          ost = ost_pool.tile([128, 512], F32, tag="ost")
                          nc.vector.tensor_copy(ost[:], pso[:])
                          nc.sync.dma_start(
                              out_d[m * 128:(m + 1) * 128, half * 512:(half + 1) * 512],
                              ost[:])
    nc.compile()
    return nc


def _make_masks():
    # tri0[p, c] = 0 if c >= p else -1e10 (lower-triangle additive mask for
    # diagonal chunks whose moving window starts at the chunk diagonal)
    p = np.arange(128)[:, None]
    c = np.arange(512)[None, :]
    return np.where(c >= p, 0.0, -1e10).astype(np.float32)


def _make_in_maps(x, w_qkv, w_out):
    masks = _make_masks()
    scale = np.float32(DH ** -0.5)
    in_maps = []
    for c in range(NCORES):
        g = c % 2
        wq = w_qkv[:, g * 512:(g + 1) * 512] * scale
        wk = w_qkv[:, D + g * 512: D + (g + 1) * 512]
        wv = w_qkv[:, 2 * D + g * 512: 2 * D + (g + 1) * 512]
        in_maps.append({
            "x": np.ascontiguousarray(x[c // 2]),
            "wqkv": np.ascontiguousarray(np.concatenate([wq, wk, wv], axis=1)),
            "wout": np.ascontiguousarray(w_out[g * 512:(g + 1) * 512, :]),
            "masks": masks,
        })
    return in_maps


def kernel(x, w_qkv, w_out):
    from concourse.bass_utils import run_bass_kernel_spmd

    x = np.asarray(x, dtype=np.float32)
    w_qkv = np.asarray(w_qkv, dtype=np.float32)
    w_out = np.asarray(w_out, dtype=np.float32)
    assert x.shape == (B, S, D) and w_qkv.shape == (D, 3 * D) and w_out.shape == (D, D)

    if "nc" not in _CACHE:
        _CACHE["nc"] = _build_nc()
    nc = _CACHE["nc"]

    in_maps = _make_in_maps(x, w_qkv, w_out)
    res = run_bass_kernel_spmd(nc, in_maps, core_ids=list(range(NCORES)),
                               trace=False)
    out = np.empty((B, S, D), dtype=np.float32)
    for b in range(B):
        out[b] = res.results[2 * b]["out"] + res.results[2 * b + 1]["out"]
    return out


# revision 6
# speedup vs baseline: 1.0744x; 1.0039x over previous
"""Causal self-attention on 8 trn2 NeuronCores.

Sharding (per the batch+head hint): core c handles batch b = c//2 (data
parallel) and head-group g = c%2 (8 of 16 heads; tensor-parallel slice of
w_qkv columns / w_out rows). Each core computes a full-batch-slice partial
of the output projection over its 512 head dims; the two partials per batch
are summed on gather (the "all-reduce after out_proj").

Kernel dataflow per core (S=2048 tokens, D=1024, 8 heads x 64):
  phase 1: x^T via PE transposes; qkvT = Wslice^T @ x^T (fp32r matmuls,
           512-token blocks so N=512 hides the fp32 LDWEIGHTS)
           -> qT, kT in bf16 [64h, S] layouts; v transposed back to natural
           [S, 64] bf16 with an appended ones column (denominator trick).
  phase 2: per head, exact-causal flash attention in transposed orientation,
           all bf16 on the PE: scoresT[k,q] = kT_chunk.T @ qT (scale folded
           into Wq on host), single lower-triangle additive mask on the
           diagonal chunks, exp with no max subtraction (scores are O(N(0,1)),
           fp32-safe), out_hT[dh,q] += [v|1].T @ probsT where the ones row
           accumulates the softmax denominator; normalize via reciprocal +
           gpsimd partition_broadcast.
  phase 3: partial out = oT.T @ Wout_slice (fp32r) per 128-token chunk.
"""
import numpy as np

B = 4
S = 2048
D = 1024
HG = 8           # heads per core
DH = 64
NCORES = 8
NB = S // 512    # 512-token blocks in phase 1
KC = D // 128    # contraction chunks over D
MC = (3 * 512) // 128  # qkv output chunks per core (12)

_CACHE = {}


def _build_nc():
    import concourse.bass as bass  # noqa
    import concourse.mybir as mybir
    import concourse.tile as tile
    from concourse import bacc
    from concourse.masks import make_identity

    F32 = mybir.dt.float32
    FR = mybir.dt.float32r
    BF = mybir.dt.bfloat16
    Exp = mybir.ActivationFunctionType.Exp

    nc = bacc.Bacc("TRN2", target_bir_lowering=False, debug=False,
                   enable_asserts=False, num_devices=NCORES)
    x_d = nc.dram_tensor("x", [S, D], F32, kind="ExternalInput")
    wqkv_d = nc.dram_tensor("wqkv", [D, 3 * 512], FR, kind="ExternalInput")
    wout_d = nc.dram_tensor("wout", [512, D], FR, kind="ExternalInput")
    masks_d = nc.dram_tensor("masks", [128, 512], F32, kind="ExternalInput")
    out_d = nc.dram_tensor("out", [S, D], F32, kind="ExternalOutput")

    with tile.TileContext(nc) as tc:
        with tc.tile_pool(name="persist", bufs=1) as persist:
            qT = persist.tile([128, 4 * S], BF)
            kT = persist.tile([128, 4 * S], BF)
            v1 = persist.tile([128, HG * 16 * 65], BF)
            ident = persist.tile([128, 128], F32)
            make_identity(nc, ident[:])
            ones128 = persist.tile([128, 128], F32)
            nc.gpsimd.memset(ones128[:], 1.0)
            tri0 = persist.tile([128, 512], F32)
            nc.sync.dma_start(tri0[:], masks_d.ap())
            # ones column of every [*, 65] v chunk
            nc.scalar.copy(
                v1[:].rearrange("p (c u) -> p c u", u=65)[:, :, 64:65],
                ones128[:].rearrange("p (c u) -> p c u", u=1),
            )

            # ---------------- phase 1: qkv projection ----------------
            with tc.tile_pool(name="wqkv", bufs=1) as wq_pool, \
                 tc.tile_pool(name="xin", bufs=2) as xin_pool, \
                 tc.tile_pool(name="xT", bufs=2) as xT_pool, \
                 tc.tile_pool(name="vst", bufs=2) as vst_pool, \
                 tc.tile_pool(name="ps_tp", bufs=2, space="PSUM") as ps_tp, \
                 tc.tile_pool(name="ps_mm", bufs=3, space="PSUM") as ps_mm, \
                 tc.tile_pool(name="ps_vt", bufs=2, space="PSUM") as ps_vt:
                w_sb = wq_pool.tile([128, KC * 1536], FR)
                nc.sync.dma_start(
                    w_sb[:].rearrange("p (k n) -> p k n", k=KC),
                    wqkv_d.ap().rearrange("(k p) n -> p k n", p=128),
                )
                for tb in range(NB):
                    xT_sb = xT_pool.tile([128, KC * 512], FR, tag="xT")
                    x_tiles = []
                    for t in range(4):
                        x_sb = xin_pool.tile([128, D], F32, tag=f"x{t}")
                        nc.sync.dma_start(
                            x_sb[:], x_d[(tb * 4 + t) * 128:(tb * 4 + t + 1) * 128, :])
                        x_tiles.append(x_sb)
                    for ki in range(KC):
                        tp = ps_tp.tile([128, 512], F32, tag="tp")
                        for t in range(4):
                            nc.tensor.transpose(
                                tp[:, t * 128:(t + 1) * 128],
                                x_tiles[t][:, ki * 128:(ki + 1) * 128], ident[:])
                        nc.vector.tensor_copy(
                            xT_sb[:, ki * 512:(ki + 1) * 512], tp[:])
                    for m in range(MC):
                        acc = ps_mm.tile([128, 512], F32, tag="acc")
                        for ki in range(KC):
                            nc.tensor.matmul(
                                acc[:],
                                w_sb[:, ki * 1536 + m * 128: ki * 1536 + (m + 1) * 128],
                                xT_sb[:, ki * 512:(ki + 1) * 512],
                                start=(ki == 0), stop=(ki == KC - 1))
                        if m < 4:
                            nc.scalar.copy(
                                qT[:, m * S + tb * 512: m * S + tb * 512 + 512], acc[:])
                        elif m < 8:
                            nc.scalar.copy(
                                kT[:, (m - 4) * S + tb * 512: (m - 4) * S + tb * 512 + 512],
                                acc[:])
                        else:
                            vst = vst_pool.tile([128, 512], F32, tag="vst")
                            nc.vector.tensor_copy(vst[:], acc[:])
                            for t in range(4):
                                pvt = ps_vt.tile([128, 128], F32, tag="pvt")
                                nc.tensor.transpose(
                                    pvt[:], vst[:, t * 128:(t + 1) * 128], ident[:])
                                sck = tb * 4 + t
                                h0 = 2 * (m - 8)
                                nc.vector.tensor_copy(
                                    v1[:, (h0 * 16 + sck) * 65:(h0 * 16 + sck) * 65 + 64],
                                    pvt[:, 0:64])
                                nc.vector.tensor_copy(
                                    v1[:, ((h0 + 1) * 16 + sck) * 65:((h0 + 1) * 16 + sck) * 65 + 64],
                                    pvt[:, 64:128])

            # ---------------- phases 2+3 tensors ----------------
            with tc.tile_pool(name="attnbig", bufs=1) as attnbig:
              oT = attnbig.tile([128, 4 * S], FR)
              # ---------------- phase 2: causal attention ----------------
              with tc.tile_pool(name="probs", bufs=14) as pr_pool, \
                   tc.tile_pool(name="recip", bufs=2) as rc_pool, \
                   tc.tile_pool(name="rbc", bufs=2) as rb_pool, \
                   tc.tile_pool(name="ps_sc", bufs=3, space="PSUM") as ps_sc, \
                   tc.tile_pool(name="ps_out", bufs=5, space="PSUM") as ps_out:
                  for h in range(HG):
                      r = h // 2
                      po = 64 * (h % 2)
                      out_tiles = [
                          ps_out.tile([65, 512], F32, tag="oj", name=f"outps_h{h}_j{j}")
                          for j in range(4)
                      ]
                      probs_tiles = {}

                      def emit_scores(s):
                          for j in range(s // 4, 4):
                              lo = max(128 * s - 512 * j, 0)
                              n = 512 - lo
                              scp = ps_sc.tile([128, 512], F32, tag="sc",
                                               name=f"sc_h{h}_s{s}_j{j}")
                              nc.tensor.matmul(
                                  scp[:, :n],
                                  kT[po:po + 64, r * S + s * 128: r * S + s * 128 + 128],
                                  qT[po:po + 64, r * S + 512 * j + lo: r * S + 512 * (j + 1)],
                                  start=True, stop=True)
                              if 128 * s >= 512 * j:
                                  nc.vector.tensor_add(
                                      scp[:, :128], scp[:, :128], tri0[:, :128])
                              pr = pr_pool.tile([128, 512], BF, tag="probs",
                                                name=f"pr_h{h}_s{s}_j{j}")
                              nc.scalar.activation(pr[:, :n], scp[:, :n], Exp)
                              probs_tiles[(s, j)] = pr

                      def normalize(j):
                          rc = rc_pool.tile([1, 512], F32, tag="rc")
                          nc.vector.reciprocal(rc[:], out_tiles[j][64:65, :])
                          rb = rb_pool.tile([64, 512], F32, tag="rb")
                          nc.gpsimd.partition_broadcast(rb[:], rc[:])
                          nc.vector.tensor_mul(
                              oT[po:po + 64, r * S + 512 * j: r * S + 512 * j + 512],
                              out_tiles[j][0:64, :], rb[:])

                      def emit_out(s):
                          for j in range(s // 4, 4):
                              lo = max(128 * s - 512 * j, 0)
                              n = 512 - lo
                              pr = probs_tiles.pop((s, j))
                              nc.tensor.matmul(
                                  out_tiles[j][:, lo:512],
                                  v1[:, (h * 16 + s) * 65:(h * 16 + s) * 65 + 65],
                                  pr[:, :n],
                                  start=(s == 0), stop=(s == 4 * j + 3))
                              if s == 4 * j + 3:
                                  normalize(j)

                      emit_scores(0)
                      emit_scores(1)
                      for s in range(2, 16):
                          emit_scores(s)
                          emit_out(s - 2)
                      emit_out(14)
                      emit_out(15)

              # ---------------- phase 3: output projection ----------------
              with tc.tile_pool(name="wout", bufs=1) as wo_pool, \
                   tc.tile_pool(name="ostage", bufs=4) as ost_pool, \
                   tc.tile_pool(name="ps_o", bufs=4, space="PSUM") as ps_o:
                  wout_sb = wo_pool.tile([128, 4 * D], FR)
                  nc.sync.dma_start(
                      wout_sb[:].rearrange("p (k n) -> p k n", k=4),
                      wout_d.ap().rearrange("(k p) n -> p k n", p=128),
                  )
                  for m in range(S // 128):
                      for half in range(2):
                          pso = ps_o.tile([128, 512], F32, tag="pso")
                          for k in range(4):
                              nc.tensor.matmul(
                                  pso[:],
                                  oT[:, k * S + m * 128: k * S + m * 128 + 128],
                                  wout_sb[:, k * D + half * 512: k * D + half * 512 + 512],
                                  start=(k == 0), stop=(k == 3))
                          ost = ost_pool.tile([128, 512], F32, tag="ost")
                          nc.vector.tensor_copy(ost[:], pso[:])
                          nc.sync.dma_start(
                              out_d[m * 128:(m + 1) * 128, half * 512:(half + 1) * 512],
                              ost[:])
    nc.compile()
    return nc


def _make_masks():
    # tri0[p, c] = 0 if c >= p else -1e10 (lower-triangle additive mask for
    # diagonal chunks whose moving window starts at the chunk diagonal)
    p = np.arange(128)[:, None]
    c = np.arange(512)[None, :]
    return np.where(c >= p, 0.0, -1e10).astype(np.float32)


def _make_in_maps(x, w_qkv, w_out):
    masks = _make_masks()
    scale = np.float32(DH ** -0.5)
    in_maps = []
    for c in range(NCORES):
        g = c % 2
        wq = w_qkv[:, g * 512:(g + 1) * 512] * scale
        wk = w_qkv[:, D + g * 512: D + (g + 1) * 512]
        wv = w_qkv[:, 2 * D + g * 512: 2 * D + (g + 1) * 512]
        in_maps.append({
            "x": np.ascontiguousarray(x[c // 2]),
            "wqkv": np.ascontiguousarray(np.concatenate([wq, wk, wv], axis=1)),
            "wout": np.ascontiguousarray(w_out[g * 512:(g + 1) * 512, :]),
            "masks": masks,
        })
    return in_maps


def kernel(x, w_qkv, w_out):
    from concourse.bass_utils import run_bass_kernel_spmd

    x = np.asarray(x, dtype=np.float32)
    w_qkv = np.asarray(w_qkv, dtype=np.float32)
    w_out = np.asarray(w_out, dtype=np.float32)
    assert x.shape == (B, S, D) and w_qkv.shape == (D, 3 * D) and w_out.shape == (D, D)

    if "nc" not in _CACHE:
        _CACHE["nc"] = _build_nc()
    nc = _CACHE["nc"]

    in_maps = _make_in_maps(x, w_qkv, w_out)
    res = run_bass_kernel_spmd(nc, in_maps, core_ids=list(range(NCORES)),
                               trace=False)
    out = np.empty((B, S, D), dtype=np.float32)
    for b in range(B):
        out[b] = res.results[2 * b]["out"] + res.results[2 * b + 1]["out"]
    return out
